# revision 11
# baseline (speedup 1.0000x reference)
"""DeepFM forward kernel for 8 Trainium2 NeuronCores (Bass/Tile), v3.

Strategy (data-parallel over batch, per the sharding hint):
  - Batch B=16384 split 8 ways -> 2048 rows/core; tables + weights
    replicated.
  - Host builds, per field, a [size_f, 256]-bf16 table whose rows are
    [emb_row(128) | fc_value | zeros]. Transposed SWDGE dma_gathers
    yield the FEATURE-MAJOR activation tiles embT[e, b] directly (plus
    the fc value on partition 0 of the second 128-block) -- no PE
    transposes and half the gather traffic of an f32 gather.
  - Gathers are chunked per j-tile (512 batch columns) so the Q7
    descriptor-generation cost (~15 ns/row, the gather bottleneck)
    pipelines under the PE's matmul stream instead of serializing in
    front of it.
  - FM row stats via ones-vector matmuls (partition-dim reduction on
    the PE, f32 PSUM accumulate); the global-scalar partials are
    written out as gpart (1 float/core, summed on host = the only
    collective).
  - MLP runs feature-major in fp8 (E4M3) with DoubleRow perf mode:
    weights host-cast to fp8 in the interleaved [ki, (g ko), m]
    layout, activations produced by the scalar engine directly in the
    paired [128, 2, b] layout, so every 256-wide contraction group is
    ONE matmul (2x effective PE throughput vs bf16).
  - Layer 4 (512 -> 1) and the fc linear term share one [1, NB] PSUM
    accumulation group; ypre = mlp_pre + lin goes to DRAM.
  - Phase B is a trivial kernel: y = sigmoid(ypre + S) with
    S = bias + b4 + 0.5 * sum(gpart) folded on host.
"""

import os
import numpy as np
import ml_dtypes

# ---- problem constants (hardcoded; kernel.py must be self-contained) ----
TOTAL = 38279
CAT_SIZES = [31360, 6807, 18, 94]
EMB = 128
F = 4
B = 16384
N_CORES = 8
P = 128
NB = 512                       # matmul moving width (batch columns)
OFFSETS_NP = np.array([0, 31360, 38167, 38185], dtype=np.int32)

_build_cache = {}


def _build_a(b_loc, n_cores):
    """Phase A: chunked gathers + FM partials + fp8 MLP -> ypre, gpart."""
    import concourse.bass as bass  # noqa: F401
    import concourse.mybir as mybir
    import concourse.tile as tile
    from concourse import bacc, library_config

    f32 = mybir.dt.float32
    bf16 = mybir.dt.bfloat16
    fp8 = mybir.dt.float8e4
    i16 = mybir.dt.int16
    AF = mybir.ActivationFunctionType
    ALU = mybir.AluOpType
    AX = mybir.AxisListType
    DR = mybir.MatmulPerfMode.DoubleRow

    NJ = b_loc // NB             # j-tiles
    NIXC = NB // 16              # idx tile free dim per (field, chunk)

    nc = bacc.Bacc(
        "TRN2",
        target_bir_lowering=False,
        debug=False,
        num_devices=n_cores,
    )

    # ---- DRAM I/O ----
    tabs = [
        nc.dram_tensor(f"tab{f}", [CAT_SIZES[f], 256], bf16,
                       kind="ExternalInput").ap()
        for f in range(F)
    ]
    # all (field, chunk) idx tiles packed in one tensor: [128, F*NJ*NIXC]
    ix_d = nc.dram_tensor("ix", [P, F * NJ * NIXC], i16,
                          kind="ExternalInput").ap()
    w1q_d = nc.dram_tensor("w1q", [P, 4, 2048], fp8, kind="ExternalInput").ap()
    w2q_d = nc.dram_tensor("w2q", [P, 16, 1024], fp8, kind="ExternalInput").ap()
    w3q_d = nc.dram_tensor("w3q", [P, 8, 512], fp8, kind="ExternalInput").ap()
    w4q_d = nc.dram_tensor("w4q", [P, 4], fp8, kind="ExternalInput").ap()
    b1p_d = nc.dram_tensor("b1p", [P, 16], f32, kind="ExternalInput").ap()
    b2p_d = nc.dram_tensor("b2p", [P, 8], f32, kind="ExternalInput").ap()
    b3p_d = nc.dram_tensor("b3p", [P, 4], f32, kind="ExternalInput").ap()
    ypre_d = nc.dram_tensor("ypre", [1, b_loc], f32, kind="ExternalOutput").ap()
    gpart_d = nc.dram_tensor("gpart", [1, 1], f32, kind="ExternalOutput").ap()

    with tile.TileContext(nc) as tc:
        with (
            tc.tile_pool(name="const", bufs=1) as const,
            tc.tile_pool(name="gat", bufs=2) as gat,
            tc.tile_pool(name="work", bufs=2) as work,
            tc.tile_pool(name="psmm", bufs=3, space="PSUM") as psum_mm,
            tc.tile_pool(name="psfm", bufs=2, space="PSUM") as psum_fm,
            tc.tile_pool(name="psl4", bufs=1, space="PSUM") as psum_l4,
        ):
            # dma_gather ucode lives in the gpsimd "mlp" library
            nc.gpsimd.load_library(library_config.mlp)

            # ---- idx tiles first (gathers depend on them), then weights ----
            ones_col = const.tile([P, 1], bf16, tag="ones_col")
            nc.vector.memset(ones_col[:], 1.0)
            ix_sb = const.tile([P, F * NJ * NIXC], i16, tag="ix_sb")
            nc.sync.dma_start(ix_sb[:], ix_d)
            w1q = const.tile([P, 4, 2048], fp8, tag="w1q")
            nc.sync.dma_start(w1q[:], w1q_d)
            b1p = const.tile([P, 16], f32, tag="b1p")
            nc.sync.dma_start(b1p[:], b1p_d)
            b2p = const.tile([P, 8], f32, tag="b2p")
            nc.sync.dma_start(b2p[:], b2p_d)
            b3p = const.tile([P, 4], f32, tag="b3p")
            nc.sync.dma_start(b3p[:], b3p_d)
            w4q = const.tile([P, 4], fp8, tag="w4q")
            nc.sync.dma_start(w4q[:], w4q_d)
            w2q = const.tile([P, 16, 1024], fp8, tag="w2q")
            nc.sync.dma_start(w2q[:], w2q_d)
            w3q = const.tile([P, 8, 512], fp8, tag="w3q")
            nc.sync.dma_start(w3q[:], w3q_d)

            ypre_sb = const.tile([1, b_loc], f32, tag="ypre_sb")
            gacc = const.tile([1, NB], f32, tag="gacc")
            nc.vector.memset(gacc[:], 0.0)

            def ixsl(f, j):
                k = (j * F + f) * NIXC
                return ix_sb[:, k:k + NIXC]

            for j in range(NJ):
                jsl = slice(j * NB, (j + 1) * NB)
                # ---- chunked transposed gathers: [e, s, b] per field ----
                Gj = [gat.tile([P, 2, NB], bf16, tag=f"g{f}", name=f"g{f}_{j}")
                      for f in range(F)]
                for f in range(F):
                    nc.gpsimd.dma_gather(
                        Gj[f][:], tabs[f], ixsl(f, j), NB, NB, 256,
                        transpose=True, single_packet=False,
                    )
                # fp8 DoubleRow pair tiles (L1 rhs) + bf16 squares (FM)
                PTj = [gat.tile([P, 2, NB], fp8, tag=f"p{g}", name=f"p{g}_{j}")
                       for g in range(2)]
                SQj = [gat.tile([P, NB], bf16, tag=f"sq{f}", name=f"sq{f}_{j}")
                       for f in range(F)]
                for f in range(F):
                    nc.vector.tensor_copy(PTj[f // 2][:, f % 2, :],
                                          Gj[f][:, 0, :])
                    nc.vector.tensor_tensor(
                        out=SQj[f][:], in0=Gj[f][:, 0, :], in1=Gj[f][:, 0, :],
                        op=ALU.mult,
                    )

                # ---- FM: rowsumT / rowssqT on the PE ----
                psA = psum_fm.tile([1, NB], f32, tag="psA")
                for f in range(F):
                    nc.tensor.matmul(
                        psA[:], lhsT=ones_col[:], rhs=Gj[f][:, 0, :],
                        start=(f == 0), stop=(f == F - 1),
                    )
                psB = psum_fm.tile([1, NB], f32, tag="psB")
                for f in range(F):
                    nc.tensor.matmul(
                        psB[:], lhsT=ones_col[:], rhs=SQj[f][:],
                        start=(f == 0), stop=(f == F - 1),
                    )
                rs = work.tile([1, NB], f32, tag="rs", name=f"rs{j}")
                nc.vector.tensor_copy(rs[:], psA[:])
                dd = work.tile([1, NB], f32, tag="dd", name=f"dd{j}")
                nc.vector.tensor_tensor(out=dd[:], in0=rs[:], in1=rs[:],
                                        op=ALU.mult)
                nc.vector.tensor_tensor(out=dd[:], in0=dd[:], in1=psB[:],
                                        op=ALU.subtract)
                nc.vector.tensor_tensor(out=gacc[:], in0=gacc[:], in1=dd[:],
                                        op=ALU.add)

                # ---- MLP layers 1-3, fp8 DoubleRow, feature-major ----
                H1j = [work.tile([P, 2, NB], fp8, tag=f"h1_{g}",
                                 name=f"h1_{g}_{j}") for g in range(8)]
                H2j = [work.tile([P, 2, NB], fp8, tag=f"h2_{g}",
                                 name=f"h2_{g}_{j}") for g in range(4)]
                H3j = [work.tile([P, 2, NB], fp8, tag=f"h3_{g}",
                                 name=f"h3_{g}_{j}") for g in range(2)]
                layers = [
                    (2, 16, w1q, b1p, PTj, H1j),
                    (8, 8, w2q, b2p, H1j, H2j),
                    (4, 4, w3q, b3p, H2j, H3j),
                ]
                for (KG, MT, wq, bp, Hin, Hout) in layers:
                    for mt in range(MT):
                        ps = psum_mm.tile([P, NB], f32, tag="mm")
                        for g in range(KG):
                            nc.tensor.matmul(
                                ps[:],
                                lhsT=wq[:, 2 * g:2 * g + 2,
                                        mt * P:(mt + 1) * P],
                                rhs=Hin[g][:],
                                start=(g == 0), stop=(g == KG - 1),
                                perf_mode=DR,
                            )
                        nc.scalar.activation(
                            Hout[mt // 2][:, mt % 2, :], ps[:], AF.Relu,
                            bias=bp[:, mt:mt + 1],
                        )

                # ---- L4 (512->1) + fc linear term in one PSUM group ----
                ps4 = psum_l4.tile([1, NB], f32, tag="l4")
                for kt in range(4):
                    nc.tensor.matmul(
                        ps4[:], lhsT=w4q[:, kt:kt + 1],
                        rhs=H3j[kt // 2][:, kt % 2, :],
                        start=(kt == 0), stop=False,
                    )
                for f in range(F):
                    nc.tensor.matmul(
                        ps4[:], lhsT=ones_col[:], rhs=Gj[f][:, 1, :],
                        start=False, stop=(f == F - 1),
                    )
                nc.scalar.activation(ypre_sb[:, jsl], ps4[:], AF.Identity)

            # ---- outputs ----
            gp = const.tile([1, 1], f32, tag="gp")
            nc.vector.reduce_sum(out=gp[:], in_=gacc[:], axis=AX.X)
            nc.sync.dma_start(gpart_d, gp[:])
            nc.sync.dma_start(ypre_d, ypre_sb[:])

    nc.compile()
    return nc


def _build_b(b_loc, n_cores):
    """Phase B: y = sigmoid(ypre + S), S folded on host."""
    import concourse.mybir as mybir
    import concourse.tile as tile
    from concourse import bacc

    f32 = mybir.dt.float32
    AF = mybir.ActivationFunctionType
    NCH = b_loc // P

    nc = bacc.Bacc(
        "TRN2",
        target_bir_lowering=False,
        debug=False,
        num_devices=n_cores,
    )
    yin_d = nc.dram_tensor("yin", [P, NCH], f32, kind="ExternalInput").ap()
    sv_d = nc.dram_tensor("sv", [P, 1], f32, kind="ExternalInput").ap()
    y_d = nc.dram_tensor("y", [b_loc, 1], f32, kind="ExternalOutput").ap()

    with tile.TileContext(nc) as tc:
        with tc.tile_pool(name="const", bufs=1) as const:
            yin = const.tile([P, NCH], f32, tag="yin")
            nc.sync.dma_start(yin[:], yin_d)
            sv = const.tile([P, 1], f32, tag="sv")
            nc.sync.dma_start(sv[:], sv_d)
            ysb = const.tile([P, NCH], f32, tag="ysb")
            nc.scalar.activation(ysb[:], yin[:], AF.Sigmoid, bias=sv[:])
            nc.sync.dma_start(y_d.rearrange("(c p) o -> p (c o)", p=P), ysb[:])

    nc.compile()
    return nc


def _get_program(phase, b_loc, n_cores):
    key = (phase, b_loc, n_cores)
    if key not in _build_cache:
        _build_cache[key] = (
            _build_a(b_loc, n_cores) if phase == "A" else _build_b(b_loc, n_cores)
        )
    return _build_cache[key]


def _wrap_idx(lin_idx):
    """lin_idx [n] int -> [128, n//16] int16 dma_gather index tile:
    tile[p, s] = lin_idx[s*16 + p%16] (16-wrap, replicated for 8 Q7 cores)."""
    n = lin_idx.shape[0]
    wrap = lin_idx.astype(np.int16).reshape(n // 16, 16).T  # [16, n//16]
    return np.ascontiguousarray(np.tile(wrap, (8, 1)))


def _prep_shared(inputs):
    """Host-side table/weight prep shared by all cores."""
    bf = ml_dtypes.bfloat16
    f8 = ml_dtypes.float8_e4m3
    emb16 = np.asarray(inputs["emb_table"], np.float32).astype(bf)  # [T,128]
    fc16 = np.asarray(inputs["fc"], np.float32).astype(bf)          # [T,1]
    tabs = {}
    for f in range(F):
        sz = CAT_SIZES[f]
        off = int(OFFSETS_NP[f])
        t = np.zeros((sz, 256), dtype=bf)
        t[:, :EMB] = emb16[:sz]
        t[:, EMB] = fc16[off:off + sz, 0]
        tabs[f"tab{f}"] = t

    def dr_pack(w, kgroups):
        # [K, M] f32 -> [128, 2*kgroups, M] fp8 with (ki, (g ko), m) layout
        K, M = w.shape
        w = np.asarray(w, np.float32).reshape(kgroups, 2, P, M)
        return np.ascontiguousarray(
            w.transpose(2, 0, 1, 3).reshape(P, 2 * kgroups, M).astype(f8)
        )

    sh = dict(tabs)
    sh["w1q"] = dr_pack(np.asarray(inputs["W1"]), 2)
    sh["w2q"] = dr_pack(np.asarray(inputs["W2"]), 8)
    sh["w3q"] = dr_pack(np.asarray(inputs["W3"]), 4)
    sh["w4q"] = np.ascontiguousarray(
        np.asarray(inputs["W4"], np.float32).reshape(4, P).T.astype(f8)
    )
    for name, mt in (("b1", 16), ("b2", 8), ("b3", 4)):
        sh[f"{name}p"] = np.ascontiguousarray(
            np.asarray(inputs[name], np.float32).reshape(mt, P).T
        )
    return sh


def _pack_ix(xs):
    """Per-core [b_loc, F] ids -> [128, F*NJ*NIXC] int16 tile, chunk-major:
    block (j, f) holds _wrap_idx(ids of field f, batch chunk j)."""
    b_loc = xs.shape[0]
    NJ = b_loc // NB
    cols = []
    for j in range(NJ):
        for f in range(F):
            cols.append(_wrap_idx(xs[j * NB:(j + 1) * NB, f]))
    return np.ascontiguousarray(np.concatenate(cols, axis=1))


def kernel(**inputs) -> np.ndarray:
    from concourse.bass_utils import run_bass_kernel_spmd

    n_cores = N_CORES
    b_loc = B // n_cores
    cores = list(range(n_cores))
    trace = bool(int(os.environ.get("KERNEL_TRACE", "0")))

    x_int = np.asarray(inputs["x"], np.float32).astype(np.int32)  # [B, F]
    shared = _prep_shared(inputs)

    # Phase A: per-core ypre + FM partial
    ncA = _get_program("A", b_loc, n_cores)
    in_maps = []
    for c in range(n_cores):
        m = dict(shared)
        m["ix"] = _pack_ix(x_int[c * b_loc:(c + 1) * b_loc])
        in_maps.append(m)
    resA = run_bass_kernel_spmd(ncA, in_maps, core_ids=cores, trace=trace)

    g = np.float32(0.0)
    for r in resA.results:
        g = np.float32(g + np.float32(r["gpart"][0, 0]))
    S = np.float32(
        np.asarray(inputs["bias"], np.float32).reshape(-1)[0]
        + np.asarray(inputs["b4"], np.float32).reshape(-1)[0]
        + 0.5 * g
    )

    # Phase B: y = sigmoid(ypre + S)
    ncB = _get_program("B", b_loc, n_cores)
    sv = np.full((P, 1), S, dtype=np.float32)
    NCH = b_loc // P
    in_maps_b = []
    for c in range(n_cores):
        ypre = np.asarray(resA.results[c]["ypre"], np.float32).reshape(b_loc)
        in_maps_b.append({
            "yin": np.ascontiguousarray(ypre.reshape(NCH, P).T),
            "sv": sv,
        })
    resB = run_bass_kernel_spmd(ncB, in_maps_b, core_ids=cores, trace=trace)

    kernel._last_results = (resA, resB)
    a_ns = resA.exec_time_ns
    b_ns = resB.exec_time_ns
    kernel._last_exec_ns = (
        (a_ns or 0) + (b_ns or 0) if (a_ns is not None or b_ns is not None)
        else None
    )
    kernel._last_exec_parts = (a_ns, b_ns)
    out = np.concatenate([r["y"] for r in resB.results], axis=0)
    return out.astype(np.float32)


# revision 12
# speedup vs baseline: 1.0500x; 1.0500x over previous
"""DeepFM forward kernel for 8 Trainium2 NeuronCores (Bass/Tile), v3.

Strategy (data-parallel over batch, per the sharding hint):
  - Batch B=16384 split 8 ways -> 2048 rows/core; tables + weights
    replicated.
  - Host builds, per field, a [size_f, 256]-bf16 table whose rows are
    [emb_row(128) | fc_value | zeros]. Transposed SWDGE dma_gathers
    yield the FEATURE-MAJOR activation tiles embT[e, b] directly (plus
    the fc value on partition 0 of the second 128-block) -- no PE
    transposes and half the gather traffic of an f32 gather.
  - Gathers are chunked per j-tile (512 batch columns) so the Q7
    descriptor-generation cost (~15 ns/row, the gather bottleneck)
    pipelines under the PE's matmul stream instead of serializing in
    front of it.
  - FM row stats via ones-vector matmuls (partition-dim reduction on
    the PE, f32 PSUM accumulate); the global-scalar partials are
    written out as gpart (1 float/core, summed on host = the only
    collective).
  - MLP runs feature-major in fp8 (E4M3) with DoubleRow perf mode:
    weights host-cast to fp8 in the interleaved [ki, (g ko), m]
    layout, activations produced by the scalar engine directly in the
    paired [128, 2, b] layout, so every 256-wide contraction group is
    ONE matmul (2x effective PE throughput vs bf16).
  - Layer 4 (512 -> 1) and the fc linear term share one [1, NB] PSUM
    accumulation group; ypre = mlp_pre + lin goes to DRAM.
  - Phase B is a trivial kernel: y = sigmoid(ypre + S) with
    S = bias + b4 + 0.5 * sum(gpart) folded on host.
"""

import os
import numpy as np
import ml_dtypes

# ---- problem constants (hardcoded; kernel.py must be self-contained) ----
TOTAL = 38279
CAT_SIZES = [31360, 6807, 18, 94]
EMB = 128
F = 4
B = 16384
N_CORES = 8
P = 128
NB = 512                       # matmul moving width (batch columns)
OFFSETS_NP = np.array([0, 31360, 38167, 38185], dtype=np.int32)

_build_cache = {}


def _build_a(b_loc, n_cores):
    """Phase A: chunked gathers + FM partials + fp8 MLP -> ypre, gpart."""
    import concourse.bass as bass  # noqa: F401
    import concourse.mybir as mybir
    import concourse.tile as tile
    from concourse import bacc, library_config

    f32 = mybir.dt.float32
    bf16 = mybir.dt.bfloat16
    fp8 = mybir.dt.float8e4
    i16 = mybir.dt.int16
    AF = mybir.ActivationFunctionType
    ALU = mybir.AluOpType
    AX = mybir.AxisListType
    DR = mybir.MatmulPerfMode.DoubleRow

    NJ = b_loc // NB             # j-tiles
    NIXC = NB // 16              # idx tile free dim per (field, chunk)

    nc = bacc.Bacc(
        "TRN2",
        target_bir_lowering=False,
        debug=False,
        num_devices=n_cores,
    )

    # ---- DRAM I/O ----
    tabs = [
        nc.dram_tensor(f"tab{f}", [CAT_SIZES[f], 256], bf16,
                       kind="ExternalInput").ap()
        for f in range(F)
    ]
    # all (field, chunk) idx tiles packed in one tensor: [128, F*NJ*NIXC]
    ix_d = nc.dram_tensor("ix", [P, F * NJ * NIXC], i16,
                          kind="ExternalInput").ap()
    w1q_d = nc.dram_tensor("w1q", [P, 4, 2048], fp8, kind="ExternalInput").ap()
    w2q_d = nc.dram_tensor("w2q", [P, 16, 1024], fp8, kind="ExternalInput").ap()
    w3q_d = nc.dram_tensor("w3q", [P, 8, 512], fp8, kind="ExternalInput").ap()
    w4q_d = nc.dram_tensor("w4q", [P, 4], fp8, kind="ExternalInput").ap()
    b1p_d = nc.dram_tensor("b1p", [P, 16], f32, kind="ExternalInput").ap()
    b2p_d = nc.dram_tensor("b2p", [P, 8], f32, kind="ExternalInput").ap()
    b3p_d = nc.dram_tensor("b3p", [P, 4], f32, kind="ExternalInput").ap()
    ypre_d = nc.dram_tensor("ypre", [1, b_loc], f32, kind="ExternalOutput").ap()
    gpart_d = nc.dram_tensor("gpart", [1, 1], f32, kind="ExternalOutput").ap()

    with tile.TileContext(nc) as tc:
        with (
            tc.tile_pool(name="const", bufs=1) as const,
            tc.tile_pool(name="gat", bufs=4) as gat,
            tc.tile_pool(name="work", bufs=2) as work,
            tc.tile_pool(name="psmm", bufs=3, space="PSUM") as psum_mm,
            tc.tile_pool(name="psfm", bufs=2, space="PSUM") as psum_fm,
            tc.tile_pool(name="psl4", bufs=1, space="PSUM") as psum_l4,
        ):
            # dma_gather ucode lives in the gpsimd "mlp" library
            nc.gpsimd.load_library(library_config.mlp)

            # ---- idx tiles first (gathers depend on them), then weights ----
            ones_col = const.tile([P, 1], bf16, tag="ones_col")
            nc.vector.memset(ones_col[:], 1.0)
            ix_sb = const.tile([P, F * NJ * NIXC], i16, tag="ix_sb")
            nc.sync.dma_start(ix_sb[:], ix_d)
            w1q = const.tile([P, 4, 2048], fp8, tag="w1q")
            nc.sync.dma_start(w1q[:], w1q_d)
            b1p = const.tile([P, 16], f32, tag="b1p")
            nc.sync.dma_start(b1p[:], b1p_d)
            b2p = const.tile([P, 8], f32, tag="b2p")
            nc.sync.dma_start(b2p[:], b2p_d)
            b3p = const.tile([P, 4], f32, tag="b3p")
            nc.sync.dma_start(b3p[:], b3p_d)
            w4q = const.tile([P, 4], fp8, tag="w4q")
            nc.sync.dma_start(w4q[:], w4q_d)
            w2q = const.tile([P, 16, 1024], fp8, tag="w2q")
            nc.sync.dma_start(w2q[:], w2q_d)
            w3q = const.tile([P, 8, 512], fp8, tag="w3q")
            nc.sync.dma_start(w3q[:], w3q_d)

            ypre_sb = const.tile([1, b_loc], f32, tag="ypre_sb")
            gacc = const.tile([1, NB], f32, tag="gacc")
            nc.vector.memset(gacc[:], 0.0)

            def ixsl(f, j):
                k = (j * F + f) * NIXC
                return ix_sb[:, k:k + NIXC]

            for j in range(NJ):
                jsl = slice(j * NB, (j + 1) * NB)
                # ---- chunked transposed gathers: [e, s, b] per field ----
                Gj = [gat.tile([P, 2, NB], bf16, tag=f"g{f}", name=f"g{f}_{j}")
                      for f in range(F)]
                for f in range(F):
                    nc.gpsimd.dma_gather(
                        Gj[f][:], tabs[f], ixsl(f, j), NB, NB, 256,
                        transpose=True, single_packet=False,
                    )
                # fp8 DoubleRow pair tiles (L1 rhs) + bf16 squares (FM)
                PTj = [gat.tile([P, 2, NB], fp8, tag=f"p{g}", name=f"p{g}_{j}")
                       for g in range(2)]
                SQj = [gat.tile([P, NB], bf16, tag=f"sq{f}", name=f"sq{f}_{j}")
                       for f in range(F)]
                for f in range(F):
                    nc.vector.tensor_copy(PTj[f // 2][:, f % 2, :],
                                          Gj[f][:, 0, :])
                    nc.vector.tensor_tensor(
                        out=SQj[f][:], in0=Gj[f][:, 0, :], in1=Gj[f][:, 0, :],
                        op=ALU.mult,
                    )

                # ---- FM: rowsumT / rowssqT on the PE ----
                psA = psum_fm.tile([1, NB], f32, tag="psA")
                for f in range(F):
                    nc.tensor.matmul(
                        psA[:], lhsT=ones_col[:], rhs=Gj[f][:, 0, :],
                        start=(f == 0), stop=(f == F - 1),
                    )
                psB = psum_fm.tile([1, NB], f32, tag="psB")
                for f in range(F):
                    nc.tensor.matmul(
                        psB[:], lhsT=ones_col[:], rhs=SQj[f][:],
                        start=(f == 0), stop=(f == F - 1),
                    )
                rs = work.tile([1, NB], f32, tag="rs", name=f"rs{j}")
                nc.vector.tensor_copy(rs[:], psA[:])
                dd = work.tile([1, NB], f32, tag="dd", name=f"dd{j}")
                nc.vector.tensor_tensor(out=dd[:], in0=rs[:], in1=rs[:],
                                        op=ALU.mult)
                nc.vector.tensor_tensor(out=dd[:], in0=dd[:], in1=psB[:],
                                        op=ALU.subtract)
                nc.vector.tensor_tensor(out=gacc[:], in0=gacc[:], in1=dd[:],
                                        op=ALU.add)

                # ---- MLP layers 1-3, fp8 DoubleRow, feature-major ----
                H1j = [work.tile([P, 2, NB], fp8, tag=f"h1_{g}",
                                 name=f"h1_{g}_{j}") for g in range(8)]
                H2j = [work.tile([P, 2, NB], fp8, tag=f"h2_{g}",
                                 name=f"h2_{g}_{j}") for g in range(4)]
                H3j = [work.tile([P, 2, NB], fp8, tag=f"h3_{g}",
                                 name=f"h3_{g}_{j}") for g in range(2)]
                layers = [
                    (2, 16, w1q, b1p, PTj, H1j),
                    (8, 8, w2q, b2p, H1j, H2j),
                    (4, 4, w3q, b3p, H2j, H3j),
                ]
                for (KG, MT, wq, bp, Hin, Hout) in layers:
                    for mt in range(MT):
                        ps = psum_mm.tile([P, NB], f32, tag="mm")
                        for g in range(KG):
                            nc.tensor.matmul(
                                ps[:],
                                lhsT=wq[:, 2 * g:2 * g + 2,
                                        mt * P:(mt + 1) * P],
                                rhs=Hin[g][:],
                                start=(g == 0), stop=(g == KG - 1),
                                perf_mode=DR,
                            )
                        nc.scalar.activation(
                            Hout[mt // 2][:, mt % 2, :], ps[:], AF.Relu,
                            bias=bp[:, mt:mt + 1],
                        )

                # ---- L4 (512->1) + fc linear term in one PSUM group ----
                ps4 = psum_l4.tile([1, NB], f32, tag="l4")
                for kt in range(4):
                    nc.tensor.matmul(
                        ps4[:], lhsT=w4q[:, kt:kt + 1],
                        rhs=H3j[kt // 2][:, kt % 2, :],
                        start=(kt == 0), stop=False,
                    )
                for f in range(F):
                    nc.tensor.matmul(
                        ps4[:], lhsT=ones_col[:], rhs=Gj[f][:, 1, :],
                        start=False, stop=(f == F - 1),
                    )
                nc.scalar.activation(ypre_sb[:, jsl], ps4[:], AF.Identity)

            # ---- outputs ----
            gp = const.tile([1, 1], f32, tag="gp")
            nc.vector.reduce_sum(out=gp[:], in_=gacc[:], axis=AX.X)
            nc.sync.dma_start(gpart_d, gp[:])
            nc.sync.dma_start(ypre_d, ypre_sb[:])

    nc.compile()
    return nc


def _build_b(b_loc, n_cores):
    """Phase B: y = sigmoid(ypre + S), S folded on host."""
    import concourse.mybir as mybir
    import concourse.tile as tile
    from concourse import bacc

    f32 = mybir.dt.float32
    AF = mybir.ActivationFunctionType
    NCH = b_loc // P

    nc = bacc.Bacc(
        "TRN2",
        target_bir_lowering=False,
        debug=False,
        num_devices=n_cores,
    )
    yin_d = nc.dram_tensor("yin", [P, NCH], f32, kind="ExternalInput").ap()
    sv_d = nc.dram_tensor("sv", [P, 1], f32, kind="ExternalInput").ap()
    y_d = nc.dram_tensor("y", [b_loc, 1], f32, kind="ExternalOutput").ap()

    with tile.TileContext(nc) as tc:
        with tc.tile_pool(name="const", bufs=1) as const:
            yin = const.tile([P, NCH], f32, tag="yin")
            nc.sync.dma_start(yin[:], yin_d)
            sv = const.tile([P, 1], f32, tag="sv")
            nc.sync.dma_start(sv[:], sv_d)
            ysb = const.tile([P, NCH], f32, tag="ysb")
            nc.scalar.activation(ysb[:], yin[:], AF.Sigmoid, bias=sv[:])
            nc.sync.dma_start(y_d.rearrange("(c p) o -> p (c o)", p=P), ysb[:])

    nc.compile()
    return nc


def _get_program(phase, b_loc, n_cores):
    key = (phase, b_loc, n_cores)
    if key not in _build_cache:
        _build_cache[key] = (
            _build_a(b_loc, n_cores) if phase == "A" else _build_b(b_loc, n_cores)
        )
    return _build_cache[key]


def _wrap_idx(lin_idx):
    """lin_idx [n] int -> [128, n//16] int16 dma_gather index tile:
    tile[p, s] = lin_idx[s*16 + p%16] (16-wrap, replicated for 8 Q7 cores)."""
    n = lin_idx.shape[0]
    wrap = lin_idx.astype(np.int16).reshape(n // 16, 16).T  # [16, n//16]
    return np.ascontiguousarray(np.tile(wrap, (8, 1)))


def _prep_shared(inputs):
    """Host-side table/weight prep shared by all cores."""
    bf = ml_dtypes.bfloat16
    f8 = ml_dtypes.float8_e4m3
    emb16 = np.asarray(inputs["emb_table"], np.float32).astype(bf)  # [T,128]
    fc16 = np.asarray(inputs["fc"], np.float32).astype(bf)          # [T,1]
    tabs = {}
    for f in range(F):
        sz = CAT_SIZES[f]
        off = int(OFFSETS_NP[f])
        t = np.zeros((sz, 256), dtype=bf)
        t[:, :EMB] = emb16[:sz]
        t[:, EMB] = fc16[off:off + sz, 0]
        tabs[f"tab{f}"] = t

    def dr_pack(w, kgroups):
        # [K, M] f32 -> [128, 2*kgroups, M] fp8 with (ki, (g ko), m) layout
        K, M = w.shape
        w = np.asarray(w, np.float32).reshape(kgroups, 2, P, M)
        return np.ascontiguousarray(
            w.transpose(2, 0, 1, 3).reshape(P, 2 * kgroups, M).astype(f8)
        )

    sh = dict(tabs)
    sh["w1q"] = dr_pack(np.asarray(inputs["W1"]), 2)
    sh["w2q"] = dr_pack(np.asarray(inputs["W2"]), 8)
    sh["w3q"] = dr_pack(np.asarray(inputs["W3"]), 4)
    sh["w4q"] = np.ascontiguousarray(
        np.asarray(inputs["W4"], np.float32).reshape(4, P).T.astype(f8)
    )
    for name, mt in (("b1", 16), ("b2", 8), ("b3", 4)):
        sh[f"{name}p"] = np.ascontiguousarray(
            np.asarray(inputs[name], np.float32).reshape(mt, P).T
        )
    return sh


def _pack_ix(xs):
    """Per-core [b_loc, F] ids -> [128, F*NJ*NIXC] int16 tile, chunk-major:
    block (j, f) holds _wrap_idx(ids of field f, batch chunk j)."""
    b_loc = xs.shape[0]
    NJ = b_loc // NB
    cols = []
    for j in range(NJ):
        for f in range(F):
            cols.append(_wrap_idx(xs[j * NB:(j + 1) * NB, f]))
    return np.ascontiguousarray(np.concatenate(cols, axis=1))


def kernel(**inputs) -> np.ndarray:
    from concourse.bass_utils import run_bass_kernel_spmd

    n_cores = N_CORES
    b_loc = B // n_cores
    cores = list(range(n_cores))
    trace = bool(int(os.environ.get("KERNEL_TRACE", "0")))

    x_int = np.asarray(inputs["x"], np.float32).astype(np.int32)  # [B, F]
    shared = _prep_shared(inputs)

    # Phase A: per-core ypre + FM partial
    ncA = _get_program("A", b_loc, n_cores)
    in_maps = []
    for c in range(n_cores):
        m = dict(shared)
        m["ix"] = _pack_ix(x_int[c * b_loc:(c + 1) * b_loc])
        in_maps.append(m)
    resA = run_bass_kernel_spmd(ncA, in_maps, core_ids=cores, trace=trace)

    g = np.float32(0.0)
    for r in resA.results:
        g = np.float32(g + np.float32(r["gpart"][0, 0]))
    S = np.float32(
        np.asarray(inputs["bias"], np.float32).reshape(-1)[0]
        + np.asarray(inputs["b4"], np.float32).reshape(-1)[0]
        + 0.5 * g
    )

    # Phase B: y = sigmoid(ypre + S)
    ncB = _get_program("B", b_loc, n_cores)
    sv = np.full((P, 1), S, dtype=np.float32)
    NCH = b_loc // P
    in_maps_b = []
    for c in range(n_cores):
        ypre = np.asarray(resA.results[c]["ypre"], np.float32).reshape(b_loc)
        in_maps_b.append({
            "yin": np.ascontiguousarray(ypre.reshape(NCH, P).T),
            "sv": sv,
        })
    resB = run_bass_kernel_spmd(ncB, in_maps_b, core_ids=cores, trace=trace)

    kernel._last_results = (resA, resB)
    a_ns = resA.exec_time_ns
    b_ns = resB.exec_time_ns
    kernel._last_exec_ns = (
        (a_ns or 0) + (b_ns or 0) if (a_ns is not None or b_ns is not None)
        else None
    )
    kernel._last_exec_parts = (a_ns, b_ns)
    out = np.concatenate([r["y"] for r in resB.results], axis=0)
    return out.astype(np.float32)
